# revision 1
# baseline (speedup 1.0000x reference)
"""DGCNN part-segmentation forward pass on 8 Trainium2 NeuronCores.

Sharding: data-parallel over the batch (B=4) x 2-way split of the N=4096
points within each batch element.  Core c handles batch element c//2,
point rows [(c%2)*2048, (c%2+1)*2048).  The two cores of a pair exchange
their half of each EdgeConv output with a pairwise AllGather (and a
pairwise AllReduce-max for the global pooling feature).

Device algorithm per EdgeConv layer (transform-then-gather):
  dist  : one fused matmul  s[i,j] = 2<x_i,x_j> - |x_j|^2  (row-rank equal
          to the reference's negative squared distance)
  top-20: per 128-row tile: 16x chunk-max8 (DVE Max) -> 128 candidates,
          3 peeling rounds (max8 + match_replace) -> top-24 values,
          3x max_index over the full row -> global indices (first 20 used)
  gather: GPSIMD ap_gather of the u = U x transform (per neighbor slot k)
  edge   : psum = I*u_gathered + (V/2)*(2 x_i)  (two matmuls), then
           LeakyReLU(. + c) on the scalar engine (Prelu, alpha=0.2)
  conv2  : 64x64 matmul + LeakyReLU epilogue (layers 1, 2)
  k-max  : running tensor_max over the 20 neighbor slots (DVE)
"""

import sys

sys.path.insert(0, "/opt/trn_rl_repo")

import numpy as np

B = 4
N = 4096
H = 2048  # points per core (half of a batch element)
KNN = 20
EPS = 1e-5
NEG = -3.0e38

_CACHE = {}


# --------------------------------------------------------------------------
# host-side weight preparation
# --------------------------------------------------------------------------

def _fold_bn(w, b, g, be):
    s = (g / np.sqrt(np.float32(1.0) + np.float32(EPS))).astype(np.float32)
    return (w * s[:, None]).astype(np.float32), (s * b + be).astype(np.float32)


def _prep_weights(inp):
    f = np.float32
    W, C = {}, {}
    for i in range(1, 9):
        W[i], C[i] = _fold_bn(
            inp["w%d" % i], inp["b%d" % i], inp["g%d" % i], inp["be%d" % i]
        )
    d = {}
    # edge conv layers: split into U (neighbor part) and V (center part)
    for lid, wi in ((1, 1), (2, 3), (3, 5)):
        w = W[wi]
        cin = w.shape[1] // 2
        U = w[:, :cin]
        V = w[:, cin:] - w[:, :cin]
        d["u%dT" % lid] = np.ascontiguousarray(U.T)
        d["v%dTh" % lid] = np.ascontiguousarray((V / f(2.0)).T)
        d["c%d" % lid] = C[wi].reshape(64, 1)
    d["w2T"] = np.ascontiguousarray(W[2].T)
    d["cc2"] = C[2].reshape(64, 1)
    d["w4T"] = np.ascontiguousarray(W[4].T)
    d["cc4"] = C[4].reshape(64, 1)
    # conv6 (192 -> 1024)
    w6T = np.ascontiguousarray(W[6].T)  # (192, 1024)
    d["w6aT"] = np.ascontiguousarray(w6T[:128])
    d["w6bT"] = np.ascontiguousarray(w6T[128:])
    d["c6v"] = np.ascontiguousarray(C[6].reshape(8, 128).T)  # (128, 8)
    # conv7 (1216 -> 512): xg part (1024) and local part (192)
    w7 = W[7]
    w7gT = np.ascontiguousarray(w7[:, :1024].T)  # (1024, 512)
    d["w7gT"] = np.ascontiguousarray(w7gT.reshape(8, 128, 512).transpose(1, 0, 2))
    w7lT = np.ascontiguousarray(w7[:, 1024:].T)  # (192, 512)
    d["w7laT"] = np.ascontiguousarray(w7lT[:128])
    d["w7lbT"] = np.ascontiguousarray(w7lT[128:])
    d["c7v"] = np.ascontiguousarray(C[7].reshape(4, 128).T)  # (128, 4)
    # conv8 (512 -> 256)
    w8T = np.ascontiguousarray(W[8].T)  # (512, 256)
    d["w8T"] = np.ascontiguousarray(w8T.reshape(4, 128, 256).transpose(1, 0, 2))
    d["c8v"] = np.ascontiguousarray(C[8].reshape(2, 128).T)  # (128, 2)
    # conv9 (256 -> 13), plain linear
    w9T = np.ascontiguousarray(inp["w9"].T.astype(f))  # (256, 13)
    d["w9T"] = np.ascontiguousarray(w9T.reshape(2, 128, 13).transpose(1, 0, 2))
    d["b9v"] = inp["b9"].astype(f).reshape(13, 1)
    # constants
    d["id64"] = np.eye(64, dtype=f)
    d["ones3"] = np.ones((3, 1), dtype=f)
    d["ones64"] = np.ones((64, 1), dtype=f)
    return d


_WEIGHT_SPECS = [
    ("u1T", (3, 64)), ("v1Th", (3, 64)), ("c1", (64, 1)),
    ("w2T", (64, 64)), ("cc2", (64, 1)),
    ("u2T", (64, 64)), ("v2Th", (64, 64)), ("c2", (64, 1)),
    ("w4T", (64, 64)), ("cc4", (64, 1)),
    ("u3T", (64, 64)), ("v3Th", (64, 64)), ("c3", (64, 1)),
    ("w6aT", (128, 1024)), ("w6bT", (64, 1024)), ("c6v", (128, 8)),
    ("w7gT", (128, 8, 512)), ("w7laT", (128, 512)), ("w7lbT", (64, 512)),
    ("c7v", (128, 4)),
    ("w8T", (128, 4, 256)), ("c8v", (128, 2)),
    ("w9T", (128, 2, 13)), ("b9v", (13, 1)),
    ("id64", (64, 64)), ("ones3", (3, 1)), ("ones64", (64, 1)),
]


# --------------------------------------------------------------------------
# device kernel builder
# --------------------------------------------------------------------------

def build_kernel():
    import concourse.bacc as bacc
    import concourse.mybir as mybir
    import concourse.tile as tile

    f32 = mybir.dt.float32
    f32r = mybir.dt.float32r
    i16 = mybir.dt.int16
    AF = mybir.ActivationFunctionType

    def R(ap):
        # float32r needs fp32r-rounded producers end-to-end; disabled.
        return ap
    PAIRS = [[0, 1], [2, 3], [4, 5], [6, 7]]

    nc = bacc.Bacc("TRN2", target_bir_lowering=False, num_devices=8)

    u16 = mybir.dt.uint16
    pts_full = nc.dram_tensor("pts_full", [3, N], f32, kind="ExternalInput")
    choff_d = nc.dram_tensor("choff", [128, 128], u16, kind="ExternalInput")
    pts_mine = nc.dram_tensor("pts_mine", [3, H], f32, kind="ExternalInput")
    wdram = {}
    for name, shape in _WEIGHT_SPECS:
        wdram[name] = nc.dram_tensor(name, list(shape), f32, kind="ExternalInput")
    out_d = nc.dram_tensor("out", [13, H], f32, kind="ExternalOutput")

    with tile.TileContext(nc) as tc:
        with (
            tc.tile_pool(name="wp", bufs=1) as wp,
            tc.tile_pool(name="per", bufs=1) as per,
            tc.tile_pool(name="psd", bufs=4, space="PSUM") as psd,
            tc.tile_pool(name="pse", bufs=2, space="PSUM") as pse,
            tc.tile_pool(name="dram", bufs=1, space="DRAM") as drp,
        ):
            # ---- load weights ----
            wsb = {}
            for name, shape in _WEIGHT_SPECS:
                t = wp.tile(list(shape), f32, tag=name, name="w_" + name)
                nc.sync.dma_start(t, wdram[name][:])
                wsb[name] = t

            # ---- persistent tiles ----
            rhsF = per.tile([65, N], f32, tag="rhsF", name="rhsF")
            lhsTm = per.tile([65, H], f32, tag="lhsTm", name="lhsTm")
            u_t = per.tile([64, N], f32, tag="u", name="u_t")
            xcat_a = per.tile([128, H], f32, tag="xcata", name="xcat_a")
            xcat_b = per.tile([64, H], f32, tag="xcatb", name="xcat_b")
            widx = per.tile([64, KNN * 128], i16, tag="widx", name="widx")
            acc3 = per.tile([64, H], f32, tag="acc3", name="acc3")
            x2acc = per.tile([64, H], f32, tag="x2acc", name="x2acc")
            xg_sb = per.tile([128, 8], f32, tag="xg", name="xg_sb")
            b7_sb = per.tile([128, 4], f32, tag="b7", name="b7_sb")

            idx_dram = drp.tile([H, KNN], i16, tag="idxd", name="idx_dram")
            choff_sb = per.tile([128, 128], u16, tag="choff", name="choff_sb")
            nc.sync.dma_start(choff_sb, choff_d[:])
            zero128 = per.tile([128, 128], f32, tag="z128", name="zero128")
            nc.vector.memset(zero128, 0.0)

            x1h = xcat_a[0:64]
            x2h = xcat_a[64:128]
            x3h = xcat_b

            with (
                tc.tile_pool(name="dsb", bufs=3) as dsbp,
                tc.tile_pool(name="tk", bufs=4) as tkp,
                tc.tile_pool(name="gp", bufs=2) as gp,
            ):
                def prep_sq_and_u(cin, ones_sb, uT_sb):
                    """rhsF[64] = -sum_c rhsF[c]^2 ; u = uT.T @ rhsF[0:cin]."""
                    xsq = dsbp.tile([64, N], f32, tag="dsb", name="xsq")[0:cin]
                    nc.scalar.activation(xsq, rhsF[0:cin], AF.Square)
                    sqrow = dsbp.tile([1, N], f32, tag="dsb", name="sqrow")
                    for j in range(8):
                        sl = slice(j * 512, (j + 1) * 512)
                        pq = psd.tile([1, 512], f32, tag="d", name="pq")
                        nc.tensor.matmul(pq, R(ones_sb), R(xsq[:, sl]))
                        nc.scalar.mul(sqrow[:, sl], pq, -1.0)
                        pu = psd.tile([64, 512], f32, tag="d", name="pu")
                        nc.tensor.matmul(pu, R(uT_sb), R(rhsF[0:cin, sl]))
                        nc.scalar.copy(u_t[:, sl], pu)
                    nc.sync.dma_start(rhsF[64:65], sqrow)

                def topk_phase(cin, grp):
                    """distances + top-20 indices for 1024 rows of group grp."""
                    for t in range(grp * 8, grp * 8 + 8):
                        dsb = dsbp.tile([128, N], f32, tag="dsb", name="dsb")
                        for j in range(8):
                            sl = slice(j * 512, (j + 1) * 512)
                            pd = psd.tile([128, 512], f32, tag="d", name="pd")
                            nc.tensor.matmul(
                                pd,
                                R(lhsTm[:, t * 128 : (t + 1) * 128]),
                                R(rhsF[:, sl]),
                            )
                            nc.scalar.copy(dsb[:, sl], pd)
                        cand = tkp.tile([128, 128], f32, tag="cand", name="cand")
                        cidx = tkp.tile([128, 128], mybir.dt.uint16, tag="cidx",
                                        name="cidx")
                        for c in range(16):
                            nc.vector.max(
                                out=cand[:, c * 8 : (c + 1) * 8],
                                in_=dsb[:, c * 256 : (c + 1) * 256],
                            )
                        for c in range(16):
                            nc.vector.max_index(
                                cidx[:, c * 8 : (c + 1) * 8],
                                cand[:, c * 8 : (c + 1) * 8],
                                dsb[:, c * 256 : (c + 1) * 256],
                            )
                        # chunk-local -> global indices
                        nc.vector.tensor_add(cidx, cidx, choff_sb)
                        candw = tkp.tile([128, 128], f32, tag="candw", name="candw")
                        nc.scalar.copy(candw, cand)
                        t8 = tkp.tile([128, 24], f32, tag="t8", name="t8")
                        nc.vector.max(out=t8[:, 0:8], in_=candw)
                        nc.vector.match_replace(
                            out=candw, in_to_replace=t8[:, 0:8], in_values=candw,
                            imm_value=NEG,
                        )
                        nc.vector.max(out=t8[:, 8:16], in_=candw)
                        nc.vector.match_replace(
                            out=candw, in_to_replace=t8[:, 8:16], in_values=candw,
                            imm_value=NEG,
                        )
                        nc.vector.max(out=t8[:, 16:24], in_=candw)
                        # rank slots: mask of top-20 -> prefix-sum compaction
                        mask = tkp.tile([128, 128], f32, tag="mask", name="mask")
                        nc.vector.tensor_scalar(
                            mask, cand, t8[:, 19:20], None,
                            op0=mybir.AluOpType.is_ge,
                        )
                        cums = tkp.tile([128, 128], f32, tag="cums", name="cums")
                        nc.vector.tensor_tensor_scan(
                            cums, mask, zero128, 0.0,
                            op0=mybir.AluOpType.add, op1=mybir.AluOpType.add,
                        )
                        nc.vector.tensor_mul(cums, cums, mask)
                        nc.vector.tensor_scalar_add(cums, cums, -1.0)
                        slot = tkp.tile([128, 128], i16, tag="slot", name="slot")
                        nc.vector.tensor_copy(slot, cums)
                        sel = tkp.tile([128, 24], mybir.dt.uint16, tag="sel",
                                       name="sel")
                        nc.gpsimd.local_scatter(
                            out_ap=sel,
                            data_ap=cidx,
                            idxs_ap=slot,
                            channels=128,
                            num_elems=24,
                            num_idxs=128,
                        )
                        nc.sync.dma_start(
                            idx_dram[t * 128 : (t + 1) * 128, :],
                            sel[:, 0:KNN].bitcast(i16),
                        )
                    # wrapped-mod-16 reformat for ap_gather (group grp):
                    # widx[p, G*grp + k*64 + r] = idx_dram[1024*grp + 16*r + p, k]
                    G = KNN * 64
                    src = idx_dram[grp * 1024 : (grp + 1) * 1024, :].rearrange(
                        "(r p) k -> p k r", p=16
                    )
                    for rep in range(4):
                        dst = widx[rep * 16 : (rep + 1) * 16,
                                   grp * G : (grp + 1) * G].rearrange(
                            "p (k r) -> p k r", r=64
                        )
                        nc.sync.dma_start(dst, src)

                def edge_phase(lid, cin, vTh_sb, c_ap, w2T_sb, c2_ap, x_out,
                               grp):
                    G = KNN * 64
                    gsl = slice(grp * 1024, (grp + 1) * 1024)
                    for k in range(KNN):
                        if lid == 3 and k == 0:
                            g = acc3[:, gsl]
                        else:
                            g = gp.tile([64, 1024], f32, tag="g", name="g", bufs=3)
                        nc.gpsimd.ap_gather(
                            out_ap=g,
                            in_ap=u_t,
                            idxs_ap=widx[:, grp * G + k * 64 : grp * G + (k + 1) * 64],
                            channels=64,
                            num_elems=N,
                            d=1,
                            num_idxs=1024,
                        )
                        if lid == 3:
                            if k > 0:
                                nc.vector.tensor_max(acc3[:, gsl], acc3[:, gsl], g)
                            continue
                        y = gp.tile([64, 1024], f32, tag="y", name="y")
                        for q in range(2):
                            sl = slice(q * 512, (q + 1) * 512)
                            msl = slice(grp * 1024 + q * 512,
                                        grp * 1024 + (q + 1) * 512)
                            pe_ = pse.tile([64, 512], f32, tag="e", name="pe")
                            nc.tensor.matmul(pe_, R(wsb["id64"]), R(g[:, sl]),
                                             start=True, stop=False)
                            nc.tensor.matmul(pe_, R(vTh_sb),
                                             R(lhsTm[0:cin, msl]),
                                             start=False, stop=True)
                            nc.scalar.activation(y[:, sl], pe_, AF.Prelu,
                                                 bias=c_ap, alpha=0.2)
                        z = (x_out[:, gsl] if k == 0 else
                             gp.tile([64, 1024], f32, tag="z", name="z"))
                        for q in range(2):
                            sl = slice(q * 512, (q + 1) * 512)
                            pc = pse.tile([64, 512], f32, tag="c2", name="pc")
                            nc.tensor.matmul(pc, R(w2T_sb), R(y[:, sl]))
                            nc.scalar.activation(z[:, sl], pc, AF.Prelu,
                                                 bias=c2_ap, alpha=0.2)
                        if k > 0:
                            nc.vector.tensor_max(x_out[:, gsl], x_out[:, gsl], z)
                    if lid == 3:
                        # x3 = Lrelu(max_k(u_j) + V x_i + c)  (monotone)
                        for q in range(2):
                            sl = slice(grp * 1024 + q * 512,
                                       grp * 1024 + (q + 1) * 512)
                            pe_ = pse.tile([64, 512], f32, tag="e", name="pe")
                            nc.tensor.matmul(pe_, R(wsb["id64"]),
                                             R(acc3[:, sl]),
                                             start=True, stop=False)
                            nc.tensor.matmul(pe_, R(vTh_sb),
                                             R(lhsTm[0:cin, sl]),
                                             start=False, stop=True)
                            nc.scalar.activation(x_out[:, sl], pe_, AF.Prelu,
                                                 bias=c_ap, alpha=0.2)

                def allgather_x(x_half):
                    """x_half (64, H) -> rhsF[0:64] = full (64, N), pair AG."""
                    ccin = drp.tile([64, H], f32, tag="ccin", name="ccin")
                    nc.sync.dma_start(ccin, x_half)
                    ccout = drp.tile([128, H], f32, tag="ccout", name="ccout")
                    nc.gpsimd.collective_compute(
                        "AllGather",
                        mybir.AluOpType.bypass,
                        replica_groups=PAIRS,
                        ins=[ccin],
                        outs=[ccout],
                    )
                    nc.sync.dma_start(
                        rhsF[0:64].rearrange("c (h e) -> c h e", h=2),
                        ccout.rearrange("(h c) e -> c h e", c=64),
                    )

                # ================= layer 1 =================
                nc.vector.memset(rhsF[0:64], 0.0)
                nc.sync.dma_start(rhsF[0:3], pts_full[:])
                tmp3 = gp.tile([3, H], f32, tag="tmp3", name="tmp3", bufs=1)
                nc.sync.dma_start(tmp3, pts_mine[:])
                nc.vector.memset(lhsTm[0:64], 0.0)
                nc.scalar.mul(lhsTm[0:3], tmp3, 2.0)
                nc.vector.memset(lhsTm[64:65], 1.0)
                prep_sq_and_u(3, wsb["ones3"], wsb["u1T"])
                for grp in range(2):
                    topk_phase(3, grp)
                    edge_phase(1, 3, wsb["v1Th"], wsb["c1"], wsb["w2T"],
                               wsb["cc2"], x1h, grp)

                # ================= layer 2 =================
                allgather_x(x1h)
                nc.scalar.mul(lhsTm[0:64], x1h, 2.0)
                nc.vector.memset(lhsTm[64:65], 1.0)
                prep_sq_and_u(64, wsb["ones64"], wsb["u2T"])
                for grp in range(2):
                    topk_phase(64, grp)
                    edge_phase(2, 64, wsb["v2Th"], wsb["c2"], wsb["w4T"],
                               wsb["cc4"], x2acc, grp)
                nc.sync.dma_start(x2h, x2acc)

                # ================= layer 3 =================
                allgather_x(x2acc)
                nc.scalar.mul(lhsTm[0:64], x2acc, 2.0)
                nc.vector.memset(lhsTm[64:65], 1.0)
                prep_sq_and_u(64, wsb["ones64"], wsb["u3T"])
                for grp in range(2):
                    topk_phase(64, grp)
                    edge_phase(3, 64, wsb["v3Th"], wsb["c3"], None, None, x3h,
                               grp)

                # ================= conv6 + global max pool =================
                for ob in range(8):
                    obs = slice(ob * 128, (ob + 1) * 128)
                    xgt = tkp.tile([128, 4], f32, tag="xgt", name="xgt")
                    for q in range(4):
                        sl = slice(q * 512, (q + 1) * 512)
                        pf = psd.tile([128, 512], f32, tag="d", name="pf6")
                        nc.tensor.matmul(pf, R(wsb["w6aT"][:, obs]),
                                         R(xcat_a[:, sl]),
                                         start=True, stop=False)
                        nc.tensor.matmul(pf, R(wsb["w6bT"][:, obs]),
                                         R(xcat_b[:, sl]),
                                         start=False, stop=True)
                        h6 = gp.tile([128, 512], f32, tag="g", name="h6", bufs=3)
                        nc.scalar.activation(h6, pf, AF.Prelu,
                                             bias=wsb["c6v"][:, ob : ob + 1],
                                             alpha=0.2)
                        nc.vector.reduce_max(xgt[:, q : q + 1], h6,
                                             axis=mybir.AxisListType.X)
                    nc.vector.reduce_max(xg_sb[:, ob : ob + 1], xgt,
                                         axis=mybir.AxisListType.X)

            # layer scratch pools released here; final stage below.
            ccg_in = drp.tile([128, 8], f32, tag="ccgi", name="ccg_in")
            nc.sync.dma_start(ccg_in, xg_sb)
            ccg_out = drp.tile([128, 8], f32, tag="ccgo", name="ccg_out")
            nc.gpsimd.collective_compute(
                "AllReduce",
                mybir.AluOpType.max,
                replica_groups=PAIRS,
                ins=[ccg_in],
                outs=[ccg_out],
            )
            nc.sync.dma_start(xg_sb, ccg_out)

            # conv7 effective bias: c7 + W7g @ xg
            for ob in range(4):
                pb = psd.tile([128, 1], f32, tag="d", name="pb7")
                for kb in range(8):
                    nc.tensor.matmul(
                        pb,
                        wsb["w7gT"][:, kb, ob * 128 : (ob + 1) * 128],
                        xg_sb[:, kb : kb + 1],
                        start=(kb == 0),
                        stop=(kb == 7),
                    )
                nc.scalar.activation(b7_sb[:, ob : ob + 1], pb, AF.Identity,
                                     bias=wsb["c7v"][:, ob : ob + 1])

            with tc.tile_pool(name="fin", bufs=1) as fin:
                h7 = fin.tile([128, 4 * H], f32, tag="h7", name="h7")
                for ob in range(4):
                    obs = slice(ob * 128, (ob + 1) * 128)
                    for q in range(4):
                        sl = slice(q * 512, (q + 1) * 512)
                        pf = psd.tile([128, 512], f32, tag="d", name="pf7")
                        nc.tensor.matmul(pf, R(wsb["w7laT"][:, obs]),
                                         R(xcat_a[:, sl]),
                                         start=True, stop=False)
                        nc.tensor.matmul(pf, R(wsb["w7lbT"][:, obs]),
                                         R(xcat_b[:, sl]),
                                         start=False, stop=True)
                        nc.scalar.activation(
                            h7[:, ob * H + q * 512 : ob * H + (q + 1) * 512], pf,
                            AF.Prelu, bias=b7_sb[:, ob : ob + 1], alpha=0.2,
                        )
                h8 = fin.tile([128, 2 * H], f32, tag="h8", name="h8")
                for ob in range(2):
                    for q in range(4):
                        pf = psd.tile([128, 512], f32, tag="d", name="pf8")
                        for kb in range(4):
                            nc.tensor.matmul(
                                pf,
                                R(wsb["w8T"][:, kb, ob * 128 : (ob + 1) * 128]),
                                R(h7[:, kb * H + q * 512 : kb * H + (q + 1) * 512]),
                                start=(kb == 0),
                                stop=(kb == 3),
                            )
                        nc.scalar.activation(
                            h8[:, ob * H + q * 512 : ob * H + (q + 1) * 512], pf,
                            AF.Prelu, bias=wsb["c8v"][:, ob : ob + 1], alpha=0.2,
                        )
                o_sb = fin.tile([13, H], f32, tag="osb", name="o_sb")
                for q in range(4):
                    sl = slice(q * 512, (q + 1) * 512)
                    pf = psd.tile([13, 512], f32, tag="d", name="pf9")
                    for kb in range(2):
                        nc.tensor.matmul(
                            pf,
                            R(wsb["w9T"][:, kb, :]),
                            R(h8[:, kb * H + q * 512 : kb * H + (q + 1) * 512]),
                            start=(kb == 0),
                            stop=(kb == 1),
                        )
                    nc.scalar.activation(o_sb[:, sl], pf, AF.Identity,
                                         bias=wsb["b9v"])
                nc.sync.dma_start(out_d[:], o_sb)

    nc.compile()
    return nc


def make_in_maps(inputs):
    """Per-core input dicts from the full problem inputs."""
    wd = _prep_weights(inputs)
    pts = np.asarray(inputs["points"], dtype=np.float32)
    in_maps = []
    for c in range(8):
        b, h = c // 2, c % 2
        m = {name: np.ascontiguousarray(wd[name]) for name, _ in _WEIGHT_SPECS}
        m["choff"] = np.ascontiguousarray(
            np.tile(np.repeat(np.arange(16, dtype=np.uint16) * 256, 8), (128, 1)))
        m["pts_full"] = np.ascontiguousarray(pts[b])
        m["pts_mine"] = np.ascontiguousarray(pts[b][:, h * H : (h + 1) * H])
        in_maps.append(m)
    return in_maps


def kernel(**inputs):
    from concourse.bass_utils import run_bass_kernel_spmd

    if "nc" not in _CACHE:
        _CACHE["nc"] = build_kernel()
    nc = _CACHE["nc"]
    in_maps = make_in_maps(inputs)
    res = run_bass_kernel_spmd(nc, in_maps, core_ids=list(range(8)))
    out = np.zeros((B, 13, N), dtype=np.float32)
    for c in range(8):
        b, h = c // 2, c % 2
        out[b][:, h * H : (h + 1) * H] = res.results[c]["out"]
    return out



# revision 12
# speedup vs baseline: 1.0679x; 1.0679x over previous
"""DGCNN part-segmentation forward pass on 8 Trainium2 NeuronCores.

Sharding: data-parallel over the batch (B=4) x 2-way split of the N=4096
points within each batch element.  Core c handles batch element c//2,
point rows [(c%2)*2048, (c%2+1)*2048).  The two cores of a pair exchange
their half of each EdgeConv output with a pairwise AllGather (and a
pairwise AllReduce-max for the global pooling feature).

All heavy matmuls run in float32r (1 PE cycle/row vs 4 for fp32).  The
BIR verifier requires every tensor consumed by an fp32r matmul to be
*produced* as fp32r, so the feature pipeline keeps fp32r-typed tiles
whose producers are Activation-engine ops (which round on write), DMAs
of fp32r data, or ap_gather of already-rounded data.  DVE max-reduction
paths accumulate in plain f32 scratch and re-enter the fp32r domain
through a scalar-engine copy.

Device algorithm per EdgeConv layer (transform-then-gather):
  dist  : one fused matmul  s[i,j] = 2<x_i,x_j> - |x_j|^2  (row-rank equal
          to the reference's negative squared distance)
  top-20: per 128-row tile: 16x chunk-max8 (DVE Max) -> 128 candidates,
          3 peeling rounds -> top-24 values, chunk max_index -> global
          indices (first 20 used)
  gather: GPSIMD ap_gather of the u = U x transform (4 neighbor slots
          per call; cost is dominated by the 4096-wide source)
  edge   : psum = I*u_gathered + (V/2)*(2 x_i)  (two matmuls), then
           LeakyReLU(. + c) on the scalar engine (Prelu, alpha=0.2)
  conv2  : 64x64 matmul + LeakyReLU epilogue (layers 1, 2)
  k-max  : running tensor_max over the 20 neighbor slots (DVE)
"""

import sys

sys.path.insert(0, "/opt/trn_rl_repo")

import numpy as np

B = 4
N = 4096
H = 2048  # points per core (half of a batch element)
KNN = 20
EPS = 1e-5
NEG = -3.0e38

_CACHE = {}
USE_F32R = False  # toggle fp32r fast-path matmuls


# --------------------------------------------------------------------------
# host-side weight preparation
# --------------------------------------------------------------------------

def _fold_bn(w, b, g, be):
    s = (g / np.sqrt(np.float32(1.0) + np.float32(EPS))).astype(np.float32)
    return (w * s[:, None]).astype(np.float32), (s * b + be).astype(np.float32)


def _prep_weights(inp):
    f = np.float32
    W, C = {}, {}
    for i in range(1, 9):
        W[i], C[i] = _fold_bn(
            inp["w%d" % i], inp["b%d" % i], inp["g%d" % i], inp["be%d" % i]
        )
    d = {}
    # edge conv layers: split into U (neighbor part) and V (center part)
    for lid, wi in ((1, 1), (2, 3), (3, 5)):
        w = W[wi]
        cin = w.shape[1] // 2
        U = w[:, :cin]
        V = w[:, cin:] - w[:, :cin]
        d["u%dT" % lid] = np.ascontiguousarray(U.T)
        d["v%dTh" % lid] = np.ascontiguousarray((V / f(2.0)).T)
        d["c%d" % lid] = C[wi].reshape(64, 1)
    d["w2T"] = np.ascontiguousarray(W[2].T)
    d["cc2"] = C[2].reshape(64, 1)
    d["w4T"] = np.ascontiguousarray(W[4].T)
    d["cc4"] = C[4].reshape(64, 1)
    # conv6 (192 -> 1024)
    w6T = np.ascontiguousarray(W[6].T)  # (192, 1024)
    d["w6aT"] = np.ascontiguousarray(w6T[:128])
    d["w6bT"] = np.ascontiguousarray(w6T[128:])
    d["c6v"] = np.ascontiguousarray(C[6].reshape(8, 128).T)  # (128, 8)
    # conv7 (1216 -> 512): xg part (1024) and local part (192)
    w7 = W[7]
    w7gT = np.ascontiguousarray(w7[:, :1024].T)  # (1024, 512)
    d["w7gT"] = np.ascontiguousarray(w7gT.reshape(8, 128, 512).transpose(1, 0, 2))
    w7lT = np.ascontiguousarray(w7[:, 1024:].T)  # (192, 512)
    d["w7laT"] = np.ascontiguousarray(w7lT[:128])
    d["w7lbT"] = np.ascontiguousarray(w7lT[128:])
    d["c7v"] = np.ascontiguousarray(C[7].reshape(4, 128).T)  # (128, 4)
    # conv8 (512 -> 256)
    w8T = np.ascontiguousarray(W[8].T)  # (512, 256)
    d["w8T"] = np.ascontiguousarray(w8T.reshape(4, 128, 256).transpose(1, 0, 2))
    d["c8v"] = np.ascontiguousarray(C[8].reshape(2, 128).T)  # (128, 2)
    # conv9 (256 -> 13), plain linear
    w9T = np.ascontiguousarray(inp["w9"].T.astype(f))  # (256, 13)
    d["w9T"] = np.ascontiguousarray(w9T.reshape(2, 128, 13).transpose(1, 0, 2))
    d["b9v"] = inp["b9"].astype(f).reshape(13, 1)
    # constants
    d["id64"] = np.eye(64, dtype=f)
    d["id64f"] = np.eye(64, dtype=f)
    d["ones3"] = np.ones((3, 1), dtype=f)
    d["ones64"] = np.ones((64, 1), dtype=f)
    d["onesr"] = np.ones((1, H), dtype=f)
    return d


_WEIGHT_SPECS = [
    ("u1T", (3, 64)), ("v1Th", (3, 64)), ("c1", (64, 1)),
    ("w2T", (64, 64)), ("cc2", (64, 1)),
    ("u2T", (64, 64)), ("v2Th", (64, 64)), ("c2", (64, 1)),
    ("w4T", (64, 64)), ("cc4", (64, 1)),
    ("u3T", (64, 64)), ("v3Th", (64, 64)), ("c3", (64, 1)),
    ("w6aT", (128, 1024)), ("w6bT", (64, 1024)), ("c6v", (128, 8)),
    ("w7gT", (128, 8, 512)), ("w7laT", (128, 512)), ("w7lbT", (64, 512)),
    ("c7v", (128, 4)),
    ("w8T", (128, 4, 256)), ("c8v", (128, 2)),
    ("w9T", (128, 2, 13)), ("b9v", (13, 1)),
    ("id64", (64, 64)), ("id64f", (64, 64)),
    ("ones3", (3, 1)), ("ones64", (64, 1)),
]

# weights consumed (as lhsT) by fp32r matmuls -> fp32r dram/sbuf dtype
_F32R_WEIGHTS = {
    "u1T", "v1Th", "w2T", "u2T", "v2Th", "w4T", "u3T",
    "w6aT", "w6bT", "w7laT", "w7lbT", "w8T", "w9T",
    "id64", "ones3", "ones64",
}


# --------------------------------------------------------------------------
# device kernel builder
# --------------------------------------------------------------------------

def build_kernel():
    import concourse.bacc as bacc
    import concourse.mybir as mybir
    import concourse.tile as tile

    f32 = mybir.dt.float32
    f32r = mybir.dt.float32r if USE_F32R else mybir.dt.float32
    i16 = mybir.dt.int16
    AF = mybir.ActivationFunctionType

    def R(ap):
        # fp32r view for matmul operands (tiles are already fp32r-typed)
        return ap.bitcast(f32r) if USE_F32R else ap

    def F(ap):
        # plain-f32 view for non-matmul consumers of fp32r tiles
        return ap.bitcast(f32) if USE_F32R else ap

    PAIRS = [[0, 1], [2, 3], [4, 5], [6, 7]]

    nc = bacc.Bacc("TRN2", target_bir_lowering=False, num_devices=8)

    u16 = mybir.dt.uint16
    pts_full = nc.dram_tensor("pts_full", [3, N], f32r, kind="ExternalInput")
    choff_d = nc.dram_tensor("choff", [128, 128], u16, kind="ExternalInput")
    pts_mine = nc.dram_tensor("pts_mine", [3, H], f32r, kind="ExternalInput")
    onesr_d = nc.dram_tensor("onesr", [1, H], f32r, kind="ExternalInput")
    wdram = {}
    for name, shape in _WEIGHT_SPECS:
        dt_ = f32r if name in _F32R_WEIGHTS else f32
        wdram[name] = nc.dram_tensor(name, list(shape), dt_,
                                     kind="ExternalInput")
    out_d = nc.dram_tensor("out", [13, H], f32, kind="ExternalOutput")

    with tile.TileContext(nc) as tc:
        with (
            tc.tile_pool(name="wp", bufs=1) as wp,
            tc.tile_pool(name="per", bufs=1) as per,
            tc.tile_pool(name="psd", bufs=4, space="PSUM") as psd,
            tc.tile_pool(name="pse", bufs=2, space="PSUM") as pse,
            tc.tile_pool(name="dram", bufs=1, space="DRAM") as drp,
        ):
            # ---- load weights ----
            wsb = {}
            for name, shape in _WEIGHT_SPECS:
                dt_ = f32r if name in _F32R_WEIGHTS else f32
                t = wp.tile(list(shape), dt_, tag=name, name="w_" + name)
                nc.sync.dma_start(t, wdram[name][:])
                wsb[name] = t

            # ---- persistent tiles ----
            rhsF = per.tile([65, N], f32r, tag="rhsF", name="rhsF")
            lhsTm = per.tile([65, H], f32r, tag="lhsTm", name="lhsTm")
            u_t = per.tile([64, N], f32r, tag="u", name="u_t")
            xcat_a = per.tile([128, H], f32r, tag="xcata", name="xcat_a")
            xcat_b = per.tile([64, H], f32r, tag="xcatb", name="xcat_b")
            widx = per.tile([64, KNN * 128], i16, tag="widx", name="widx")
            acc3 = per.tile([64, H], f32, tag="acc3", name="acc3")
            xg_sb = per.tile([128, 8], f32, tag="xg", name="xg_sb")
            b7_sb = per.tile([128, 4], f32, tag="b7", name="b7_sb")

            idx_dram = drp.tile([H, KNN], i16, tag="idxd", name="idx_dram")
            choff_sb = per.tile([128, 128], u16, tag="choff", name="choff_sb")
            nc.sync.dma_start(choff_sb, choff_d[:])
            zero128 = per.tile([128, 128], f32, tag="z128", name="zero128")
            nc.vector.memset(zero128, 0.0)

            x1h = xcat_a[0:64]
            x2h = xcat_a[64:128]
            x3h = xcat_b

            with (
                tc.tile_pool(name="dsb", bufs=2) as dsbp,
                tc.tile_pool(name="tk", bufs=4) as tkp,
                tc.tile_pool(name="gp", bufs=2) as gp,
            ):
                def prep_sq_and_u(cin, ones_sb, uT_sb):
                    """rhsF[cin] = -sum_c rhsF[c]^2 ; u = uT.T @ rhsF[0:cin];
                    lhsTm[cin] = 1."""
                    xsq = dsbp.tile([64, N], f32r, tag="dsb", name="xsq")[0:cin]
                    nc.scalar.activation(xsq, F(rhsF[0:cin]), AF.Square)
                    sqrow = dsbp.tile([1, N], f32r, tag="dsb", name="sqrow")
                    for j in range(8):
                        sl = slice(j * 512, (j + 1) * 512)
                        pq = psd.tile([1, 512], f32, tag="d", name="pq")
                        nc.tensor.matmul(pq, R(ones_sb), R(xsq[:, sl]))
                        nc.scalar.mul(sqrow[:, sl], pq, -1.0)
                        pu = psd.tile([64, 512], f32, tag="d", name="pu")
                        nc.tensor.matmul(pu, R(uT_sb), R(rhsF[0:cin, sl]))
                        nc.scalar.copy(u_t[:, sl], pu)
                    nc.sync.dma_start(rhsF[cin : cin + 1], sqrow)
                    nc.sync.dma_start(lhsTm[cin : cin + 1], onesr_d[:])

                def topk_phase(cin, grp):
                    """distances + top-20 indices for 1024 rows of group grp."""
                    for t in range(grp * 8, grp * 8 + 8):
                        dsb = dsbp.tile([128, N], f32, tag="dsb", name="dsb")
                        for j in range(8):
                            sl = slice(j * 512, (j + 1) * 512)
                            pd = psd.tile([128, 512], f32, tag="d", name="pd")
                            nc.tensor.matmul(
                                pd,
                                R(lhsTm[0 : cin + 1, t * 128 : (t + 1) * 128]),
                                R(rhsF[0 : cin + 1, sl]),
                            )
                            nc.scalar.copy(dsb[:, sl], pd)
                        cand = tkp.tile([128, 128], f32, tag="cand", name="cand")
                        cidx = tkp.tile([128, 128], mybir.dt.uint16, tag="cidx",
                                        name="cidx")
                        for c in range(16):
                            nc.vector.max(
                                out=cand[:, c * 8 : (c + 1) * 8],
                                in_=dsb[:, c * 256 : (c + 1) * 256],
                            )
                        for c in range(16):
                            nc.vector.max_index(
                                cidx[:, c * 8 : (c + 1) * 8],
                                cand[:, c * 8 : (c + 1) * 8],
                                dsb[:, c * 256 : (c + 1) * 256],
                            )
                        # chunk-local -> global indices
                        nc.vector.tensor_add(cidx, cidx, choff_sb)
                        candw = tkp.tile([128, 128], f32, tag="candw", name="candw")
                        nc.scalar.copy(candw, cand)
                        t8 = tkp.tile([128, 24], f32, tag="t8", name="t8")
                        nc.vector.max(out=t8[:, 0:8], in_=candw)
                        nc.vector.match_replace(
                            out=candw, in_to_replace=t8[:, 0:8], in_values=candw,
                            imm_value=NEG,
                        )
                        nc.vector.max(out=t8[:, 8:16], in_=candw)
                        nc.vector.match_replace(
                            out=candw, in_to_replace=t8[:, 8:16], in_values=candw,
                            imm_value=NEG,
                        )
                        nc.vector.max(out=t8[:, 16:24], in_=candw)
                        # rank slots: mask of top-20 -> prefix-sum compaction
                        mask = tkp.tile([128, 128], f32, tag="mask", name="mask")
                        nc.vector.tensor_scalar(
                            mask, cand, t8[:, 19:20], None,
                            op0=mybir.AluOpType.is_ge,
                        )
                        cums = tkp.tile([128, 128], f32, tag="cums", name="cums")
                        nc.vector.tensor_tensor_scan(
                            cums, mask, zero128, 0.0,
                            op0=mybir.AluOpType.add, op1=mybir.AluOpType.add,
                        )
                        nc.vector.tensor_mul(cums, cums, mask)
                        nc.vector.tensor_scalar_add(cums, cums, -1.0)
                        slot = tkp.tile([128, 128], i16, tag="slot", name="slot")
                        nc.vector.tensor_copy(slot, cums)
                        sel = tkp.tile([128, 24], mybir.dt.uint16, tag="sel",
                                       name="sel")
                        nc.gpsimd.local_scatter(
                            out_ap=sel,
                            data_ap=cidx,
                            idxs_ap=slot,
                            channels=128,
                            num_elems=24,
                            num_idxs=128,
                        )
                        nc.sync.dma_start(
                            idx_dram[t * 128 : (t + 1) * 128, :],
                            sel[:, 0:KNN].bitcast(i16),
                        )
                    # wrapped-mod-16 reformat for ap_gather (group grp):
                    # widx[p, G*grp + k*64 + r] = idx_dram[1024*grp + 16*r + p, k]
                    G = KNN * 64
                    src = idx_dram[grp * 1024 : (grp + 1) * 1024, :].rearrange(
                        "(r p) k -> p k r", p=16
                    )
                    for rep in range(4):
                        dst = widx[rep * 16 : (rep + 1) * 16,
                                   grp * G : (grp + 1) * G].rearrange(
                            "p (k r) -> p k r", r=64
                        )
                        nc.sync.dma_start(dst, src)

                def edge_phase(lid, cin, vTh_sb, c_ap, w2T_sb, c2_ap, x_out,
                               grp):
                    G = KNN * 64
                    gsl = slice(grp * 1024, (grp + 1) * 1024)
                    xacc = (None if lid == 3 else
                            gp.tile([64, 1024], f32, tag="xacc", name="xacc"))
                    for kb in range(5):  # 4 neighbor slots per gather call
                        g4 = gp.tile([64, 4096], f32r, tag="g4", name="g4",
                                     bufs=2)
                        nc.gpsimd.ap_gather(
                            out_ap=g4,
                            in_ap=u_t,
                            idxs_ap=widx[:, grp * G + kb * 256
                                         : grp * G + (kb + 1) * 256],
                            channels=64,
                            num_elems=N,
                            d=1,
                            num_idxs=4096,
                        )
                        for s in range(4):
                            k = kb * 4 + s
                            g = g4[:, s * 1024 : (s + 1) * 1024]
                            if lid == 3:
                                if k == 0:
                                    nc.vector.tensor_copy(acc3[:, gsl], F(g))
                                else:
                                    nc.vector.tensor_max(acc3[:, gsl],
                                                         acc3[:, gsl], F(g))
                                continue
                            y = gp.tile([64, 1024], f32r, tag="y", name="y")
                            for q in range(2):
                                sl = slice(q * 512, (q + 1) * 512)
                                msl = slice(grp * 1024 + q * 512,
                                            grp * 1024 + (q + 1) * 512)
                                pe_ = pse.tile([64, 512], f32, tag="e", name="pe")
                                nc.tensor.matmul(pe_, R(wsb["id64"]), R(g[:, sl]),
                                                 start=True, stop=False)
                                nc.tensor.matmul(pe_, R(vTh_sb),
                                                 R(lhsTm[0:cin, msl]),
                                                 start=False, stop=True)
                                nc.scalar.activation(y[:, sl], pe_, AF.Prelu,
                                                     bias=c_ap, alpha=0.2)
                            z = (xacc if k == 0 else
                                 gp.tile([64, 1024], f32, tag="z", name="z"))
                            for q in range(2):
                                sl = slice(q * 512, (q + 1) * 512)
                                pc = pse.tile([64, 512], f32, tag="c2", name="pc")
                                nc.tensor.matmul(pc, R(w2T_sb), R(y[:, sl]))
                                nc.scalar.activation(z[:, sl], pc, AF.Prelu,
                                                     bias=c2_ap, alpha=0.2)
                            if k > 0:
                                nc.vector.tensor_max(xacc, xacc, z)
                    if lid == 3:
                        # x3 = Lrelu(max_k(u_j) + V x_i + c)  (monotone);
                        # plain-f32 matmuls (acc3 is DVE-accumulated f32)
                        for q in range(2):
                            sl = slice(grp * 1024 + q * 512,
                                       grp * 1024 + (q + 1) * 512)
                            pe_ = pse.tile([64, 512], f32, tag="e", name="pe")
                            nc.tensor.matmul(pe_, wsb["id64f"],
                                             acc3[:, sl],
                                             start=True, stop=False)
                            nc.tensor.matmul(pe_, wsb["v3Th"],
                                             F(lhsTm[0:cin, sl]),
                                             start=False, stop=True)
                            nc.scalar.activation(x_out[:, sl], pe_, AF.Prelu,
                                                 bias=c_ap, alpha=0.2)
                    else:
                        # re-enter the fp32r domain via the scalar engine
                        nc.scalar.copy(x_out[:, gsl], xacc)

                def allgather_x(x_half):
                    """x_half (64, H) -> rhsF[0:64] = full (64, N), pair AG."""
                    ccin = drp.tile([64, H], f32r, tag="ccin", name="ccin")
                    nc.sync.dma_start(ccin, x_half)
                    ccout = drp.tile([128, H], f32r, tag="ccout", name="ccout")
                    nc.gpsimd.collective_compute(
                        "AllGather",
                        mybir.AluOpType.bypass,
                        replica_groups=PAIRS,
                        ins=[F(ccin[:])],
                        outs=[F(ccout[:])],
                    )
                    nc.sync.dma_start(
                        rhsF[0:64].rearrange("c (h e) -> c h e", h=2),
                        ccout.rearrange("(h c) e -> c h e", c=64),
                    )

                # ================= layer 1 =================
                nc.sync.dma_start(rhsF[0:3], pts_full[:])
                nc.sync.dma_start(lhsTm[32:35], pts_mine[:])
                nc.scalar.mul(lhsTm[0:3], F(lhsTm[32:35]), 2.0)
                prep_sq_and_u(3, wsb["ones3"], wsb["u1T"])
                for grp in range(2):
                    topk_phase(3, grp)
                    edge_phase(1, 3, wsb["v1Th"], wsb["c1"], wsb["w2T"],
                               wsb["cc2"], x1h, grp)

                # ================= layer 2 =================
                allgather_x(x1h)
                nc.scalar.mul(lhsTm[0:64], F(x1h), 2.0)
                prep_sq_and_u(64, wsb["ones64"], wsb["u2T"])
                for grp in range(2):
                    topk_phase(64, grp)
                    edge_phase(2, 64, wsb["v2Th"], wsb["c2"], wsb["w4T"],
                               wsb["cc4"], x2h, grp)

                # ================= layer 3 =================
                allgather_x(x2h)
                nc.scalar.mul(lhsTm[0:64], F(x2h), 2.0)
                prep_sq_and_u(64, wsb["ones64"], wsb["u3T"])
                for grp in range(2):
                    topk_phase(64, grp)
                    edge_phase(3, 64, wsb["v3Th"], wsb["c3"], None, None, x3h,
                               grp)

                # ================= conv6 + global max pool =================
                for ob in range(8):
                    obs = slice(ob * 128, (ob + 1) * 128)
                    xgt = tkp.tile([128, 4], f32, tag="xgt", name="xgt")
                    for q in range(4):
                        sl = slice(q * 512, (q + 1) * 512)
                        pf = psd.tile([128, 512], f32, tag="d", name="pf6")
                        nc.tensor.matmul(pf, R(wsb["w6aT"][:, obs]),
                                         R(xcat_a[:, sl]),
                                         start=True, stop=False)
                        nc.tensor.matmul(pf, R(wsb["w6bT"][:, obs]),
                                         R(xcat_b[:, sl]),
                                         start=False, stop=True)
                        h6 = gp.tile([128, 512], f32, tag="g", name="h6", bufs=2)
                        nc.scalar.activation(h6, pf, AF.Prelu,
                                             bias=wsb["c6v"][:, ob : ob + 1],
                                             alpha=0.2)
                        nc.vector.reduce_max(xgt[:, q : q + 1], h6,
                                             axis=mybir.AxisListType.X)
                    nc.vector.reduce_max(xg_sb[:, ob : ob + 1], xgt,
                                         axis=mybir.AxisListType.X)

            # layer scratch pools released here; final stage below.
            ccg_in = drp.tile([128, 8], f32, tag="ccgi", name="ccg_in")
            nc.sync.dma_start(ccg_in, xg_sb)
            ccg_out = drp.tile([128, 8], f32, tag="ccgo", name="ccg_out")
            nc.gpsimd.collective_compute(
                "AllReduce",
                mybir.AluOpType.max,
                replica_groups=PAIRS,
                ins=[ccg_in],
                outs=[ccg_out],
            )
            nc.sync.dma_start(xg_sb, ccg_out)

            # conv7 effective bias: c7 + W7g @ xg  (plain f32 matmuls, tiny)
            for ob in range(4):
                pb = psd.tile([128, 1], f32, tag="d", name="pb7")
                for kb in range(8):
                    nc.tensor.matmul(
                        pb,
                        wsb["w7gT"][:, kb, ob * 128 : (ob + 1) * 128],
                        xg_sb[:, kb : kb + 1],
                        start=(kb == 0),
                        stop=(kb == 7),
                    )
                nc.scalar.activation(b7_sb[:, ob : ob + 1], pb, AF.Identity,
                                     bias=wsb["c7v"][:, ob : ob + 1])

            with tc.tile_pool(name="fin", bufs=1) as fin:
                h7 = fin.tile([128, 4 * H], f32r, tag="h7", name="h7")
                for ob in range(4):
                    obs = slice(ob * 128, (ob + 1) * 128)
                    for q in range(4):
                        sl = slice(q * 512, (q + 1) * 512)
                        pf = psd.tile([128, 512], f32, tag="d", name="pf7")
                        nc.tensor.matmul(pf, R(wsb["w7laT"][:, obs]),
                                         R(xcat_a[:, sl]),
                                         start=True, stop=False)
                        nc.tensor.matmul(pf, R(wsb["w7lbT"][:, obs]),
                                         R(xcat_b[:, sl]),
                                         start=False, stop=True)
                        nc.scalar.activation(
                            h7[:, ob * H + q * 512 : ob * H + (q + 1) * 512], pf,
                            AF.Prelu, bias=b7_sb[:, ob : ob + 1], alpha=0.2,
                        )
                h8 = fin.tile([128, 2 * H], f32r, tag="h8", name="h8")
                for ob in range(2):
                    for q in range(4):
                        pf = psd.tile([128, 512], f32, tag="d", name="pf8")
                        for kb in range(4):
                            nc.tensor.matmul(
                                pf,
                                R(wsb["w8T"][:, kb, ob * 128 : (ob + 1) * 128]),
                                R(h7[:, kb * H + q * 512 : kb * H + (q + 1) * 512]),
                                start=(kb == 0),
                                stop=(kb == 3),
                            )
                        nc.scalar.activation(
                            h8[:, ob * H + q * 512 : ob * H + (q + 1) * 512], pf,
                            AF.Prelu, bias=wsb["c8v"][:, ob : ob + 1], alpha=0.2,
                        )
                o_sb = fin.tile([13, H], f32, tag="osb", name="o_sb")
                for q in range(4):
                    sl = slice(q * 512, (q + 1) * 512)
                    pf = psd.tile([13, 512], f32, tag="d", name="pf9")
                    for kb in range(2):
                        nc.tensor.matmul(
                            pf,
                            R(wsb["w9T"][:, kb, :]),
                            R(h8[:, kb * H + q * 512 : kb * H + (q + 1) * 512]),
                            start=(kb == 0),
                            stop=(kb == 1),
                        )
                    nc.scalar.activation(o_sb[:, sl], pf, AF.Identity,
                                         bias=wsb["b9v"])
                nc.sync.dma_start(out_d[:], o_sb)

    nc.compile()
    return nc


def make_in_maps(inputs):
    """Per-core input dicts from the full problem inputs."""
    wd = _prep_weights(inputs)
    pts = np.asarray(inputs["points"], dtype=np.float32)
    in_maps = []
    for c in range(8):
        b, h = c // 2, c % 2
        m = {name: np.ascontiguousarray(wd[name]) for name, _ in _WEIGHT_SPECS}
        m["choff"] = np.ascontiguousarray(
            np.tile(np.repeat(np.arange(16, dtype=np.uint16) * 256, 8), (128, 1)))
        m["pts_full"] = np.ascontiguousarray(pts[b])
        m["onesr"] = np.ones((1, H), dtype=np.float32)
        m["pts_mine"] = np.ascontiguousarray(pts[b][:, h * H : (h + 1) * H])
        in_maps.append(m)
    return in_maps


def kernel(**inputs):
    from concourse.bass_utils import run_bass_kernel_spmd

    if "nc" not in _CACHE:
        _CACHE["nc"] = build_kernel()
    nc = _CACHE["nc"]
    in_maps = make_in_maps(inputs)
    res = run_bass_kernel_spmd(nc, in_maps, core_ids=list(range(8)))
    out = np.zeros((B, 13, N), dtype=np.float32)
    for c in range(8):
        b, h = c // 2, c % 2
        out[b][:, h * H : (h + 1) * H] = res.results[c]["out"]
    return out


# revision 14
# speedup vs baseline: 1.3522x; 1.2661x over previous
"""DGCNN part-segmentation forward pass on 8 Trainium2 NeuronCores.

Sharding: data-parallel over the batch (B=4) x 2-way split of the N=4096
points within each batch element.  Core c handles batch element c//2,
point rows [(c%2)*2048, (c%2+1)*2048).  The two cores of a pair exchange
their half of each EdgeConv output with a pairwise AllGather (and a
pairwise AllReduce-max for the global pooling feature).

All heavy matmuls run in float32r (1 PE cycle/row vs 4 for fp32).  The
BIR verifier requires every tensor consumed by an fp32r matmul to be
*produced* as fp32r, so the feature pipeline keeps fp32r-typed tiles
whose producers are Activation-engine ops (which round on write), DMAs
of fp32r data, or ap_gather of already-rounded data.  DVE max-reduction
paths accumulate in plain f32 scratch and re-enter the fp32r domain
through a scalar-engine copy.

Device algorithm per EdgeConv layer (transform-then-gather):
  dist  : one fused matmul  s[i,j] = 2<x_i,x_j> - |x_j|^2  (row-rank equal
          to the reference's negative squared distance)
  top-20: per 128-row tile: 16x chunk-max8 (DVE Max) -> 128 candidates,
          3 peeling rounds -> top-24 values, chunk max_index -> global
          indices (first 20 used)
  gather: GPSIMD ap_gather of the u = U x transform (4 neighbor slots
          per call; cost is dominated by the 4096-wide source)
  edge   : psum = I*u_gathered + (V/2)*(2 x_i)  (two matmuls), then
           LeakyReLU(. + c) on the scalar engine (Prelu, alpha=0.2)
  conv2  : 64x64 matmul + LeakyReLU epilogue (layers 1, 2)
  k-max  : running tensor_max over the 20 neighbor slots (DVE)
"""

import sys

sys.path.insert(0, "/opt/trn_rl_repo")

import numpy as np

B = 4
N = 4096
H = 2048  # points per core (half of a batch element)
KNN = 20
EPS = 1e-5
NEG = -3.0e38

_CACHE = {}
USE_F32R = True  # toggle fp32r fast-path matmuls


# --------------------------------------------------------------------------
# host-side weight preparation
# --------------------------------------------------------------------------

def _fold_bn(w, b, g, be):
    s = (g / np.sqrt(np.float32(1.0) + np.float32(EPS))).astype(np.float32)
    return (w * s[:, None]).astype(np.float32), (s * b + be).astype(np.float32)


def _prep_weights(inp):
    f = np.float32
    W, C = {}, {}
    for i in range(1, 9):
        W[i], C[i] = _fold_bn(
            inp["w%d" % i], inp["b%d" % i], inp["g%d" % i], inp["be%d" % i]
        )
    d = {}
    # edge conv layers: split into U (neighbor part) and V (center part)
    for lid, wi in ((1, 1), (2, 3), (3, 5)):
        w = W[wi]
        cin = w.shape[1] // 2
        U = w[:, :cin]
        V = w[:, cin:] - w[:, :cin]
        d["u%dT" % lid] = np.ascontiguousarray(U.T)
        d["v%dTh" % lid] = np.ascontiguousarray((V / f(2.0)).T)
        d["c%d" % lid] = C[wi].reshape(64, 1)
    d["w2T"] = np.ascontiguousarray(W[2].T)
    d["cc2"] = C[2].reshape(64, 1)
    d["w4T"] = np.ascontiguousarray(W[4].T)
    d["cc4"] = C[4].reshape(64, 1)
    # conv6 (192 -> 1024)
    w6T = np.ascontiguousarray(W[6].T)  # (192, 1024)
    d["w6aT"] = np.ascontiguousarray(w6T[:128])
    d["w6bT"] = np.ascontiguousarray(w6T[128:])
    d["c6v"] = np.ascontiguousarray(C[6].reshape(8, 128).T)  # (128, 8)
    # conv7 (1216 -> 512): xg part (1024) and local part (192)
    w7 = W[7]
    w7gT = np.ascontiguousarray(w7[:, :1024].T)  # (1024, 512)
    d["w7gT"] = np.ascontiguousarray(w7gT.reshape(8, 128, 512).transpose(1, 0, 2))
    w7lT = np.ascontiguousarray(w7[:, 1024:].T)  # (192, 512)
    d["w7laT"] = np.ascontiguousarray(w7lT[:128])
    d["w7lbT"] = np.ascontiguousarray(w7lT[128:])
    d["c7v"] = np.ascontiguousarray(C[7].reshape(4, 128).T)  # (128, 4)
    # conv8 (512 -> 256)
    w8T = np.ascontiguousarray(W[8].T)  # (512, 256)
    d["w8T"] = np.ascontiguousarray(w8T.reshape(4, 128, 256).transpose(1, 0, 2))
    d["c8v"] = np.ascontiguousarray(C[8].reshape(2, 128).T)  # (128, 2)
    # conv9 (256 -> 13), plain linear
    w9T = np.ascontiguousarray(inp["w9"].T.astype(f))  # (256, 13)
    d["w9T"] = np.ascontiguousarray(w9T.reshape(2, 128, 13).transpose(1, 0, 2))
    d["b9v"] = inp["b9"].astype(f).reshape(13, 1)
    # constants
    d["id64"] = np.eye(64, dtype=f)
    d["id64f"] = np.eye(64, dtype=f)
    d["ones3"] = np.ones((3, 1), dtype=f)
    d["ones64"] = np.ones((64, 1), dtype=f)
    d["onesr"] = np.ones((1, H), dtype=f)
    return d


_WEIGHT_SPECS = [
    ("u1T", (3, 64)), ("v1Th", (3, 64)), ("c1", (64, 1)),
    ("w2T", (64, 64)), ("cc2", (64, 1)),
    ("u2T", (64, 64)), ("v2Th", (64, 64)), ("c2", (64, 1)),
    ("w4T", (64, 64)), ("cc4", (64, 1)),
    ("u3T", (64, 64)), ("v3Th", (64, 64)), ("c3", (64, 1)),
    ("w6aT", (128, 1024)), ("w6bT", (64, 1024)), ("c6v", (128, 8)),
    ("w7gT", (128, 8, 512)), ("w7laT", (128, 512)), ("w7lbT", (64, 512)),
    ("c7v", (128, 4)),
    ("w8T", (128, 4, 256)), ("c8v", (128, 2)),
    ("w9T", (128, 2, 13)), ("b9v", (13, 1)),
    ("id64", (64, 64)), ("id64f", (64, 64)),
    ("ones3", (3, 1)), ("ones64", (64, 1)),
]

# weights consumed (as lhsT) by fp32r matmuls -> fp32r dram/sbuf dtype
_F32R_WEIGHTS = {
    "u1T", "v1Th", "w2T", "u2T", "v2Th", "w4T", "u3T",
    "w6aT", "w6bT", "w7laT", "w7lbT", "w8T", "w9T",
    "id64", "ones3", "ones64",
}


# --------------------------------------------------------------------------
# device kernel builder
# --------------------------------------------------------------------------

def build_kernel():
    import concourse.bacc as bacc
    import concourse.mybir as mybir
    import concourse.tile as tile

    f32 = mybir.dt.float32
    f32r = mybir.dt.float32r if USE_F32R else mybir.dt.float32
    i16 = mybir.dt.int16
    AF = mybir.ActivationFunctionType

    def R(ap):
        # fp32r view for matmul operands (tiles are already fp32r-typed)
        return ap.bitcast(f32r) if USE_F32R else ap

    def F(ap):
        # plain-f32 view for non-matmul consumers of fp32r tiles
        return ap.bitcast(f32) if USE_F32R else ap

    PAIRS = [[0, 1], [2, 3], [4, 5], [6, 7]]

    nc = bacc.Bacc("TRN2", target_bir_lowering=False, num_devices=8)

    u16 = mybir.dt.uint16
    pts_full = nc.dram_tensor("pts_full", [3, N], f32r, kind="ExternalInput")
    choff_d = nc.dram_tensor("choff", [128, 128], u16, kind="ExternalInput")
    pts_mine = nc.dram_tensor("pts_mine", [3, H], f32r, kind="ExternalInput")
    onesr_d = nc.dram_tensor("onesr", [1, H], f32r, kind="ExternalInput")
    wdram = {}
    for name, shape in _WEIGHT_SPECS:
        dt_ = f32r if name in _F32R_WEIGHTS else f32
        wdram[name] = nc.dram_tensor(name, list(shape), dt_,
                                     kind="ExternalInput")
    out_d = nc.dram_tensor("out", [13, H], f32, kind="ExternalOutput")

    with tile.TileContext(nc) as tc:
        with (
            tc.tile_pool(name="wp", bufs=1) as wp,
            tc.tile_pool(name="per", bufs=1) as per,
            tc.tile_pool(name="psd", bufs=4, space="PSUM") as psd,
            tc.tile_pool(name="pse", bufs=2, space="PSUM") as pse,
            tc.tile_pool(name="dram", bufs=1, space="DRAM") as drp,
        ):
            # ---- load weights ----
            wsb = {}
            for name, shape in _WEIGHT_SPECS:
                dt_ = f32r if name in _F32R_WEIGHTS else f32
                t = wp.tile(list(shape), dt_, tag=name, name="w_" + name)
                nc.sync.dma_start(t, wdram[name][:])
                wsb[name] = t

            # ---- persistent tiles ----
            rhsF = per.tile([65, N], f32r, tag="rhsF", name="rhsF")
            lhsTm = per.tile([65, H], f32r, tag="lhsTm", name="lhsTm")
            u_t = per.tile([64, N], f32, tag="u", name="u_t")
            xcat_a = per.tile([128, H], f32r, tag="xcata", name="xcat_a")
            xcat_b = per.tile([64, H], f32r, tag="xcatb", name="xcat_b")
            widx = per.tile([64, KNN * 128], i16, tag="widx", name="widx")
            acc3 = per.tile([64, H], f32, tag="acc3", name="acc3")
            xg_sb = per.tile([128, 8], f32, tag="xg", name="xg_sb")
            b7_sb = per.tile([128, 4], f32, tag="b7", name="b7_sb")

            idx_dram = drp.tile([H, KNN], i16, tag="idxd", name="idx_dram")
            choff_sb = per.tile([128, 128], u16, tag="choff", name="choff_sb")
            nc.sync.dma_start(choff_sb, choff_d[:])
            zero128 = per.tile([128, 128], f32, tag="z128", name="zero128")
            nc.vector.memset(zero128, 0.0)

            x1h = xcat_a[0:64]
            x2h = xcat_a[64:128]
            x3h = xcat_b

            with (
                tc.tile_pool(name="dsb", bufs=2) as dsbp,
                tc.tile_pool(name="tk", bufs=4) as tkp,
                tc.tile_pool(name="gp", bufs=2) as gp,
            ):
                def prep_sq_and_u(cin, ones_sb, uT_sb):
                    """rhsF[cin] = -sum_c rhsF[c]^2 ; u = uT.T @ rhsF[0:cin];
                    lhsTm[cin] = 1."""
                    xsq = dsbp.tile([64, N], f32r, tag="dsb", name="xsq")[0:cin]
                    nc.scalar.activation(xsq, F(rhsF[0:cin]), AF.Square)
                    sqrow = dsbp.tile([1, N], f32r, tag="dsb", name="sqrow")
                    for j in range(8):
                        sl = slice(j * 512, (j + 1) * 512)
                        pq = psd.tile([1, 512], f32, tag="d", name="pq")
                        nc.tensor.matmul(pq, R(ones_sb), R(xsq[:, sl]))
                        nc.scalar.mul(sqrow[:, sl], pq, -1.0)
                        pu = psd.tile([64, 512], f32, tag="d", name="pu")
                        nc.tensor.matmul(pu, R(uT_sb), R(rhsF[0:cin, sl]))
                        nc.scalar.copy(u_t[:, sl], pu)
                    nc.sync.dma_start(rhsF[cin : cin + 1], sqrow)
                    nc.sync.dma_start(lhsTm[cin : cin + 1], onesr_d[:])

                def topk_phase(cin, grp):
                    """distances + top-20 indices for 1024 rows of group grp."""
                    for t in range(grp * 8, grp * 8 + 8):
                        dsb = dsbp.tile([128, N], f32, tag="dsb", name="dsb")
                        for j in range(8):
                            sl = slice(j * 512, (j + 1) * 512)
                            pd = psd.tile([128, 512], f32, tag="d", name="pd")
                            nc.tensor.matmul(
                                pd,
                                R(lhsTm[0 : cin + 1, t * 128 : (t + 1) * 128]),
                                R(rhsF[0 : cin + 1, sl]),
                            )
                            nc.scalar.copy(dsb[:, sl], pd)
                        cand = tkp.tile([128, 128], f32, tag="cand", name="cand")
                        cidx = tkp.tile([128, 128], mybir.dt.uint16, tag="cidx",
                                        name="cidx")
                        for c in range(16):
                            nc.vector.max(
                                out=cand[:, c * 8 : (c + 1) * 8],
                                in_=dsb[:, c * 256 : (c + 1) * 256],
                            )
                        for c in range(16):
                            nc.vector.max_index(
                                cidx[:, c * 8 : (c + 1) * 8],
                                cand[:, c * 8 : (c + 1) * 8],
                                dsb[:, c * 256 : (c + 1) * 256],
                            )
                        # chunk-local -> global indices
                        nc.vector.tensor_add(cidx, cidx, choff_sb)
                        candw = tkp.tile([128, 128], f32, tag="candw", name="candw")
                        nc.scalar.copy(candw, cand)
                        t8 = tkp.tile([128, 24], f32, tag="t8", name="t8")
                        nc.vector.max(out=t8[:, 0:8], in_=candw)
                        nc.vector.match_replace(
                            out=candw, in_to_replace=t8[:, 0:8], in_values=candw,
                            imm_value=NEG,
                        )
                        nc.vector.max(out=t8[:, 8:16], in_=candw)
                        nc.vector.match_replace(
                            out=candw, in_to_replace=t8[:, 8:16], in_values=candw,
                            imm_value=NEG,
                        )
                        nc.vector.max(out=t8[:, 16:24], in_=candw)
                        # rank slots: mask of top-20 -> prefix-sum compaction
                        mask = tkp.tile([128, 128], f32, tag="mask", name="mask")
                        nc.vector.tensor_scalar(
                            mask, cand, t8[:, 19:20], None,
                            op0=mybir.AluOpType.is_ge,
                        )
                        cums = tkp.tile([128, 128], f32, tag="cums", name="cums")
                        nc.vector.tensor_tensor_scan(
                            cums, mask, zero128, 0.0,
                            op0=mybir.AluOpType.add, op1=mybir.AluOpType.add,
                        )
                        nc.vector.tensor_mul(cums, cums, mask)
                        nc.vector.tensor_scalar_add(cums, cums, -1.0)
                        slot = tkp.tile([128, 128], i16, tag="slot", name="slot")
                        nc.vector.tensor_copy(slot, cums)
                        sel = tkp.tile([128, 24], mybir.dt.uint16, tag="sel",
                                       name="sel")
                        nc.gpsimd.local_scatter(
                            out_ap=sel,
                            data_ap=cidx,
                            idxs_ap=slot,
                            channels=128,
                            num_elems=24,
                            num_idxs=128,
                        )
                        nc.sync.dma_start(
                            idx_dram[t * 128 : (t + 1) * 128, :],
                            sel[:, 0:KNN].bitcast(i16),
                        )
                    # wrapped-mod-16 reformat for ap_gather (group grp):
                    # widx[p, G*grp + k*64 + r] = idx_dram[1024*grp + 16*r + p, k]
                    G = KNN * 64
                    src = idx_dram[grp * 1024 : (grp + 1) * 1024, :].rearrange(
                        "(r p) k -> p k r", p=16
                    )
                    for rep in range(4):
                        dst = widx[rep * 16 : (rep + 1) * 16,
                                   grp * G : (grp + 1) * G].rearrange(
                            "p (k r) -> p k r", r=64
                        )
                        nc.sync.dma_start(dst, src)

                def edge_phase(lid, cin, vTh_sb, c_ap, w2T_sb, c2_ap, x_out,
                               grp):
                    G = KNN * 64
                    gsl = slice(grp * 1024, (grp + 1) * 1024)
                    xacc = (None if lid == 3 else
                            gp.tile([64, 1024], f32, tag="xacc", name="xacc"))
                    for kb in range(5):  # 4 neighbor slots per gather call
                        g4 = gp.tile([64, 4096], f32, tag="g4", name="g4",
                                     bufs=2)
                        nc.gpsimd.ap_gather(
                            out_ap=F(g4),
                            in_ap=F(u_t),
                            idxs_ap=widx[:, grp * G + kb * 256
                                         : grp * G + (kb + 1) * 256],
                            channels=64,
                            num_elems=N,
                            d=1,
                            num_idxs=4096,
                        )
                        for s in range(4):
                            k = kb * 4 + s
                            g = g4[:, s * 1024 : (s + 1) * 1024]
                            if lid == 3:
                                if k == 0:
                                    nc.vector.tensor_copy(acc3[:, gsl], F(g))
                                else:
                                    nc.vector.tensor_max(acc3[:, gsl],
                                                         acc3[:, gsl], F(g))
                                continue
                            y = gp.tile([64, 1024], f32r, tag="y", name="y")
                            for q in range(2):
                                sl = slice(q * 512, (q + 1) * 512)
                                msl = slice(grp * 1024 + q * 512,
                                            grp * 1024 + (q + 1) * 512)
                                pe_ = pse.tile([64, 512], f32, tag="e", name="pe")
                                nc.tensor.matmul(pe_, wsb["id64f"], g[:, sl],
                                                 start=True, stop=False)
                                nc.tensor.matmul(pe_, R(vTh_sb),
                                                 R(lhsTm[0:cin, msl]),
                                                 start=False, stop=True)
                                nc.scalar.activation(y[:, sl], pe_, AF.Prelu,
                                                     bias=c_ap, alpha=0.2)
                            z = (xacc if k == 0 else
                                 gp.tile([64, 1024], f32, tag="z", name="z"))
                            for q in range(2):
                                sl = slice(q * 512, (q + 1) * 512)
                                pc = pse.tile([64, 512], f32, tag="c2", name="pc")
                                nc.tensor.matmul(pc, R(w2T_sb), R(y[:, sl]))
                                nc.scalar.activation(z[:, sl], pc, AF.Prelu,
                                                     bias=c2_ap, alpha=0.2)
                            if k > 0:
                                nc.vector.tensor_max(xacc, xacc, z)
                    if lid == 3:
                        # x3 = Lrelu(max_k(u_j) + V x_i + c)  (monotone);
                        # plain-f32 matmuls (acc3 is DVE-accumulated f32)
                        for q in range(2):
                            sl = slice(grp * 1024 + q * 512,
                                       grp * 1024 + (q + 1) * 512)
                            pe_ = pse.tile([64, 512], f32, tag="e", name="pe")
                            nc.tensor.matmul(pe_, wsb["id64f"],
                                             acc3[:, sl],
                                             start=True, stop=False)
                            nc.tensor.matmul(pe_, wsb["v3Th"],
                                             F(lhsTm[0:cin, sl]),
                                             start=False, stop=True)
                            nc.scalar.activation(x_out[:, sl], pe_, AF.Prelu,
                                                 bias=c_ap, alpha=0.2)
                    else:
                        # re-enter the fp32r domain via the scalar engine
                        nc.scalar.copy(x_out[:, gsl], xacc)

                def allgather_x(x_half):
                    """x_half (64, H) -> rhsF[0:64] = full (64, N), pair AG."""
                    ccin = drp.tile([64, H], f32r, tag="ccin", name="ccin")
                    nc.sync.dma_start(ccin, x_half)
                    ccout = drp.tile([128, H], f32r, tag="ccout", name="ccout")
                    nc.gpsimd.collective_compute(
                        "AllGather",
                        mybir.AluOpType.bypass,
                        replica_groups=PAIRS,
                        ins=[F(ccin[:])],
                        outs=[F(ccout[:])],
                    )
                    nc.sync.dma_start(
                        rhsF[0:64].rearrange("c (h e) -> c h e", h=2),
                        ccout.rearrange("(h c) e -> c h e", c=64),
                    )

                # ================= layer 1 =================
                nc.sync.dma_start(rhsF[0:3], pts_full[:])
                nc.sync.dma_start(lhsTm[32:35], pts_mine[:])
                nc.scalar.mul(lhsTm[0:3], F(lhsTm[32:35]), 2.0)
                prep_sq_and_u(3, wsb["ones3"], wsb["u1T"])
                for grp in range(2):
                    topk_phase(3, grp)
                    edge_phase(1, 3, wsb["v1Th"], wsb["c1"], wsb["w2T"],
                               wsb["cc2"], x1h, grp)

                # ================= layer 2 =================
                allgather_x(x1h)
                nc.scalar.mul(lhsTm[0:64], F(x1h), 2.0)
                prep_sq_and_u(64, wsb["ones64"], wsb["u2T"])
                for grp in range(2):
                    topk_phase(64, grp)
                    edge_phase(2, 64, wsb["v2Th"], wsb["c2"], wsb["w4T"],
                               wsb["cc4"], x2h, grp)

                # ================= layer 3 =================
                allgather_x(x2h)
                nc.scalar.mul(lhsTm[0:64], F(x2h), 2.0)
                prep_sq_and_u(64, wsb["ones64"], wsb["u3T"])
                for grp in range(2):
                    topk_phase(64, grp)
                    edge_phase(3, 64, wsb["v3Th"], wsb["c3"], None, None, x3h,
                               grp)

                # ================= conv6 + global max pool =================
                for ob in range(8):
                    obs = slice(ob * 128, (ob + 1) * 128)
                    xgt = tkp.tile([128, 4], f32, tag="xgt", name="xgt")
                    for q in range(4):
                        sl = slice(q * 512, (q + 1) * 512)
                        pf = psd.tile([128, 512], f32, tag="d", name="pf6")
                        nc.tensor.matmul(pf, R(wsb["w6aT"][:, obs]),
                                         R(xcat_a[:, sl]),
                                         start=True, stop=False)
                        nc.tensor.matmul(pf, R(wsb["w6bT"][:, obs]),
                                         R(xcat_b[:, sl]),
                                         start=False, stop=True)
                        h6 = gp.tile([128, 512], f32, tag="g", name="h6", bufs=2)
                        nc.scalar.activation(h6, pf, AF.Prelu,
                                             bias=wsb["c6v"][:, ob : ob + 1],
                                             alpha=0.2)
                        nc.vector.reduce_max(xgt[:, q : q + 1], h6,
                                             axis=mybir.AxisListType.X)
                    nc.vector.reduce_max(xg_sb[:, ob : ob + 1], xgt,
                                         axis=mybir.AxisListType.X)

            # layer scratch pools released here; final stage below.
            ccg_in = drp.tile([128, 8], f32, tag="ccgi", name="ccg_in")
            nc.sync.dma_start(ccg_in, xg_sb)
            ccg_out = drp.tile([128, 8], f32, tag="ccgo", name="ccg_out")
            nc.gpsimd.collective_compute(
                "AllReduce",
                mybir.AluOpType.max,
                replica_groups=PAIRS,
                ins=[ccg_in],
                outs=[ccg_out],
            )
            nc.sync.dma_start(xg_sb, ccg_out)

            # conv7 effective bias: c7 + W7g @ xg  (plain f32 matmuls, tiny)
            for ob in range(4):
                pb = psd.tile([128, 1], f32, tag="d", name="pb7")
                for kb in range(8):
                    nc.tensor.matmul(
                        pb,
                        wsb["w7gT"][:, kb, ob * 128 : (ob + 1) * 128],
                        xg_sb[:, kb : kb + 1],
                        start=(kb == 0),
                        stop=(kb == 7),
                    )
                nc.scalar.activation(b7_sb[:, ob : ob + 1], pb, AF.Identity,
                                     bias=wsb["c7v"][:, ob : ob + 1])

            with tc.tile_pool(name="fin", bufs=1) as fin:
                h7 = fin.tile([128, 4 * H], f32r, tag="h7", name="h7")
                for ob in range(4):
                    obs = slice(ob * 128, (ob + 1) * 128)
                    for q in range(4):
                        sl = slice(q * 512, (q + 1) * 512)
                        pf = psd.tile([128, 512], f32, tag="d", name="pf7")
                        nc.tensor.matmul(pf, R(wsb["w7laT"][:, obs]),
                                         R(xcat_a[:, sl]),
                                         start=True, stop=False)
                        nc.tensor.matmul(pf, R(wsb["w7lbT"][:, obs]),
                                         R(xcat_b[:, sl]),
                                         start=False, stop=True)
                        nc.scalar.activation(
                            h7[:, ob * H + q * 512 : ob * H + (q + 1) * 512], pf,
                            AF.Prelu, bias=b7_sb[:, ob : ob + 1], alpha=0.2,
                        )
                h8 = fin.tile([128, 2 * H], f32r, tag="h8", name="h8")
                for ob in range(2):
                    for q in range(4):
                        pf = psd.tile([128, 512], f32, tag="d", name="pf8")
                        for kb in range(4):
                            nc.tensor.matmul(
                                pf,
                                R(wsb["w8T"][:, kb, ob * 128 : (ob + 1) * 128]),
                                R(h7[:, kb * H + q * 512 : kb * H + (q + 1) * 512]),
                                start=(kb == 0),
                                stop=(kb == 3),
                            )
                        nc.scalar.activation(
                            h8[:, ob * H + q * 512 : ob * H + (q + 1) * 512], pf,
                            AF.Prelu, bias=wsb["c8v"][:, ob : ob + 1], alpha=0.2,
                        )
                o_sb = fin.tile([13, H], f32, tag="osb", name="o_sb")
                for q in range(4):
                    sl = slice(q * 512, (q + 1) * 512)
                    pf = psd.tile([13, 512], f32, tag="d", name="pf9")
                    for kb in range(2):
                        nc.tensor.matmul(
                            pf,
                            R(wsb["w9T"][:, kb, :]),
                            R(h8[:, kb * H + q * 512 : kb * H + (q + 1) * 512]),
                            start=(kb == 0),
                            stop=(kb == 1),
                        )
                    nc.scalar.activation(o_sb[:, sl], pf, AF.Identity,
                                         bias=wsb["b9v"])
                nc.sync.dma_start(out_d[:], o_sb)

    nc.compile()
    return nc


def make_in_maps(inputs):
    """Per-core input dicts from the full problem inputs."""
    wd = _prep_weights(inputs)
    pts = np.asarray(inputs["points"], dtype=np.float32)
    in_maps = []
    for c in range(8):
        b, h = c // 2, c % 2
        m = {name: np.ascontiguousarray(wd[name]) for name, _ in _WEIGHT_SPECS}
        m["choff"] = np.ascontiguousarray(
            np.tile(np.repeat(np.arange(16, dtype=np.uint16) * 256, 8), (128, 1)))
        m["pts_full"] = np.ascontiguousarray(pts[b])
        m["onesr"] = np.ones((1, H), dtype=np.float32)
        m["pts_mine"] = np.ascontiguousarray(pts[b][:, h * H : (h + 1) * H])
        in_maps.append(m)
    return in_maps


def kernel(**inputs):
    from concourse.bass_utils import run_bass_kernel_spmd

    if "nc" not in _CACHE:
        _CACHE["nc"] = build_kernel()
    nc = _CACHE["nc"]
    in_maps = make_in_maps(inputs)
    res = run_bass_kernel_spmd(nc, in_maps, core_ids=list(range(8)))
    out = np.zeros((B, 13, N), dtype=np.float32)
    for c in range(8):
        b, h = c // 2, c % 2
        out[b][:, h * H : (h + 1) * H] = res.results[c]["out"]
    return out
